# revision 8
# baseline (speedup 1.0000x reference)
"""GATv2 layer on 8 Trainium2 NeuronCores.

Problem (hardcoded): B=4, N=256, D=256, HEADS=8, DH=32, neg_slope=0.2.

    X = (H @ W_lin) split into heads               [B, h, N, 32]
    e = leaky_relu(Xi + Xj, 0.2) . a[h]            [B, h, N, N]
    e += ln(A0 + 1e-8);  e = -inf outside mask
    attn = softmax_j(e);  Y = attn @ X  (heads merged) @ W_out

Sharding: 8 cores = (batch b = core//2) x (head-group g = core%2, 4 heads
each).  Every core computes a full [N, D] partial of Y[b] (its 4 heads'
contribution through W_out rows g*128:(g+1)*128); host sums the two
partials per batch.  SPMD: all cores run the same program on pre-sliced
inputs (no partition-id branching).

Math trick: leaky(x) = 0.2*x + 0.8*relu(x), so with q = 0.2 * a^T X:

    e[h,i,j] = 0.8 * sum_d a[h,d]*relu(X[h,d,i]+X[h,d,j]) + q[h,i] + q[h,j]

The pairwise relu pass packs all 4 local heads' dims on the 128 SBUF
partitions (Xt[(h,d), i]) and is a single fused op per query i
(DVE tensor_scalar(add,max0) or ACT Relu with per-partition bias).  The
d-reduction is a PE matmul with a sliding-window view of a zero-padded
block-diagonal 0.8*a weight matrix, accumulating rows 4c+h for 32
query nodes c into one [128, 512] PSUM tile (PE requires out base
partition 32-aligned, so zero columns of the window produce +0 rows).

The score pipeline runs in bf16: the PE fill already used fp32r HIGH
(single-pass, bf16-precision multiplies), so bf16 storage costs no
accuracy while unlocking DVE 4x tensor_scalar, ACT 2x, FWL weight
loads, and 1 cycle/row PE streaming on the transpose/AV matmuls.
X itself is computed in full fp32 (LOW_HIGH) then rounded once.
"""

import numpy as np

try:
    import concourse.bass as bass
except ImportError:  # pragma: no cover - fallback for bare containers
    import sys

    sys.path.insert(0, "/opt/trn_rl_repo")
    import concourse.bass as bass

import concourse.mybir as mybir
import concourse.tile as tile
from concourse import masks
from concourse.bass_utils import run_bass_kernel_spmd

F32 = mybir.dt.float32
BF16 = mybir.dt.bfloat16
U8 = mybir.dt.uint8
AF = mybir.ActivationFunctionType
ALU = mybir.AluOpType

N = 256
D = 256
HEADS = 8
DH = 32
HL = 4  # heads per core
P = 128
NCORES = 8

# Per-c engine assignment for the pairwise relu pass.  bf16 costs:
# DVE tensor_scalar 4x ~127ns, ACT Relu 2x ~293ns (free=256), so ACT
# carries ~10/32 of the c values given its other work (exp, drains).
_ACT_C = {0, 3, 6, 10, 13, 16, 20, 23, 26, 29}


def _gen_engine(c):
    if c in _ACT_C:
        return "act"
    return "dve"


def _split_multiwait(nc, maxw=1):
    """Walrus codegen here rejects instructions with >1 sem wait ("Too many
    sync wait commands", CoreV3GenImpl setupSyncWait).  Tile's kernel-tail
    drain carries one wait per ticked processor; hoist the extras into
    single-wait NoOps on the same engine just before the instruction."""
    import bass_rust

    n = 0
    for f in nc.m.functions:
        for b in f.blocks:
            new, changed = [], False
            for i in b.instructions:
                si = i.sync_info
                ow = list(si.on_wait) if (si is not None and si.on_wait) else []
                if len(ow) > maxw:
                    extra, keep = ow[:-maxw], ow[-maxw:]
                    for w in extra:
                        nop = mybir.InstNoOp(name=f"I-waitsplit-{n}")
                        n += 1
                        nop.engine = i.engine
                        nop.sync_info = bass_rust.SyncInfo(on_wait=[w], on_update=[])
                        new.append(nop)
                    i.sync_info = bass_rust.SyncInfo(
                        on_wait=keep,
                        on_update=list(si.on_update) if si.on_update else [],
                    )
                    changed = True
                new.append(i)
            if changed:
                b.instructions = new


def build_module():
    nc = bass.Bass("TRN2", target_bir_lowering=False, debug=False)

    hb = nc.dram_tensor("Hb", [N, D], F32, kind="ExternalInput").ap()
    wlg = nc.dram_tensor("WlinG", [D, P], F32, kind="ExternalInput").ap()
    wog = nc.dram_tensor("WoutG", [P, D], F32, kind="ExternalInput").ap()
    ag = nc.dram_tensor("aG", [HL, DH], F32, kind="ExternalInput").ap()
    mask_d = nc.dram_tensor("mask", [N, N], U8, kind="ExternalInput").ap()
    a0_d = nc.dram_tensor("A0", [N, N], F32, kind="ExternalInput").ap()
    out_d = nc.dram_tensor("out", [N, D], F32, kind="ExternalOutput").ap()

    with tile.TileContext(nc) as tc:
        _body(nc, tc, hb, wlg, wog, ag, mask_d, a0_d, out_d)
    return nc


def _body(nc, tc, hb, wlg, wog, ag, mask_d, a0_d, out_d):
    from contextlib import ExitStack

    ctx = ExitStack()
    with ctx:
        const = ctx.enter_context(tc.tile_pool(name="const", bufs=1))
        work = ctx.enter_context(tc.tile_pool(name="work", bufs=3))
        spool = ctx.enter_context(tc.tile_pool(name="spool", bufs=12))
        drpool = ctx.enter_context(tc.tile_pool(name="drpool", bufs=3))
        ps = ctx.enter_context(tc.tile_pool(name="ps", bufs=4, space="PSUM"))
        fillps = ctx.enter_context(tc.tile_pool(name="fillps", bufs=3, space="PSUM"))

        # ---------------- setup: loads -------------------------------
        ident = const.tile([P, P], F32, name="ident", tag="ident")
        masks.make_identity(nc, ident[:])
        identb = const.tile([P, P], BF16, name="identb", tag="identb")
        nc.vector.tensor_copy(identb[:], ident[:])

        # HAM warmup: PE sits idle during the input DMAs, which leaves the
        # clock gate at 4/8 (1.2 GHz) for the first ~3.4us of real matmuls.
        # Nine F=128 fp32 matmuls (~430ns each cold) burn the activity
        # window so the fill pipeline starts at 8/8.
        for k in range(9):
            wrm = fillps.tile([P, 2 * N], F32, name="wrm", tag="fill")
            nc.tensor.matmul(
                wrm[:, :P], lhsT=ident[:], rhs=ident[:], start=True, stop=True
            )

        hbt = [const.tile([P, D], F32, name=f"hbt{k}", tag=f"hbt{k}") for k in range(2)]
        for k in range(2):
            nc.sync.dma_start(out=hbt[k][:], in_=hb[k * P : (k + 1) * P, :])
        wlt = [const.tile([P, P], F32, name=f"wlt{k}", tag=f"wlt{k}") for k in range(2)]
        for k in range(2):
            nc.sync.dma_start(out=wlt[k][:], in_=wlg[k * P : (k + 1) * P, :])
        wot = const.tile([P, D], F32, name="wot", tag="wot")
        nc.sync.dma_start(out=wot[:], in_=wog[:, :])
        mskt = [const.tile([P, N], U8, name=f"mskt{k}", tag=f"mskt{k}") for k in range(2)]
        a0t = [const.tile([P, N], F32, name=f"a0t{k}", tag=f"a0t{k}") for k in range(2)]
        for k in range(2):
            nc.sync.dma_start(out=mskt[k][:], in_=mask_d[k * P : (k + 1) * P, :])
            nc.sync.dma_start(out=a0t[k][:], in_=a0_d[k * P : (k + 1) * P, :])

        # Zbig: [128, 192] zeros with 0.8*aG[h] block at rows h*32, col 32+32h.
        # Sliding window Zbig[:, 32-c : 160-c] as matmul lhsT puts head h's
        # reduction of query c at output partition h*32 + c.
        # Ablk: [128, 4] blockdiag(a) for the q matmul
        ablk = const.tile([P, HL], F32, name="ablk", tag="ablk")
        nc.gpsimd.memset(ablk[:], 0.0)
        for h in range(HL):
            nc.sync.dma_start(
                out=ablk[h * DH : (h + 1) * DH, h : h + 1],
                in_=ag[h : h + 1, :],
            )
        ablkb = const.tile([P, HL], BF16, name="ablkb", tag="ablkb")
        nc.vector.tensor_copy(ablkb[:], ablk[:])
        zt = const.tile([P, 192], BF16, name="zt", tag="zt")
        nc.gpsimd.memset(zt[:], 0.0)
        nc.vector.tensor_scalar(
            out=zt[:, DH : DH + HL * DH : DH],
            in0=ablkb[:],
            scalar1=0.8,
            scalar2=None,
            op0=ALU.mult,
        )

        wotb = const.tile([P, D], BF16, name="wotb", tag="wotb")
        nc.vector.tensor_copy(wotb[:], wot[:])

        ones_t = const.tile([1, P], BF16, name="ones_t", tag="ones_t")
        nc.gpsimd.memset(ones_t[:], 1.0)
        eps_col = const.tile([P, 1], F32, name="eps_col", tag="eps_col")
        nc.gpsimd.memset(eps_col[:], 1e-8)

        # ---------------- HT = Hb^T, Xp = Hb @ WlinG (fp32), Xt bf16 -
        ht = [const.tile([P, N], F32, name=f"ht{k}", tag=f"ht{k}") for k in range(2)]
        for cb in range(2):  # column block of Hb = partition block of HT
            for ib in range(2):
                tp = ps.tile([P, N], F32, name="ps_t", tag="ps_t")
                nc.tensor.transpose(
                    tp[:, :P], hbt[ib][:, cb * P : (cb + 1) * P], ident[:]
                )
                nc.scalar.copy(ht[cb][:, ib * P : (ib + 1) * P], tp[:, :P])

        xpb = [const.tile([P, P], BF16, name=f"xpb{ib}", tag=f"xpb{ib}") for ib in range(2)]
        for ib in range(2):
            xps = ps.tile([P, N], F32, name="ps_t", tag="ps_t")
            for k in range(2):
                nc.tensor.matmul(
                    xps[:, :P],
                    lhsT=ht[k][:, ib * P : (ib + 1) * P],
                    rhs=wlt[k][:],
                    start=(k == 0),
                    stop=(k == 1),
                )
            nc.scalar.copy(xpb[ib][:], xps[:, :P])

        xtb = const.tile([P, N], BF16, name="xtb", tag="xtb")
        for ib in range(2):
            tpb = ps.tile([P, N], BF16, name="ps_t", tag="ps_t")
            nc.tensor.transpose(tpb[:, :P], xpb[ib][:], identb[:])
            nc.scalar.copy(xtb[:, ib * P : (ib + 1) * P], tpb[:, :P])

        # fp32 image of X^T: per-partition scalar operands (DVE scalar1 /
        # ACT bias) must be fp32; values are identical to the bf16 xtb.
        xtf = const.tile([P, N], F32, name="xtf", tag="xtf")
        nc.vector.tensor_copy(xtf[:], xtb[:])

        # ---------------- q = 0.2 * a^T X  --------------------------
        qps = ps.tile([HL, N], F32, name="ps_q", tag="ps_t")
        nc.tensor.matmul(qps[:], lhsT=ablkb[:], rhs=xtb[:], start=True, stop=True)
        q_sb = const.tile([HL, N], BF16, name="q_sb", tag="q_sb")
        nc.scalar.activation(q_sb[:], qps[:], AF.Copy, bias=0.0, scale=0.2)

        # q broadcast along partitions (q_j along free), per head
        qrow = [const.tile([1, N], BF16, name=f"qrow{h}", tag=f"qrow{h}") for h in range(HL)]
        for h in range(HL):
            nc.sync.dma_start(out=qrow[h][:], in_=q_sb[h : h + 1, :])
        qb = [const.tile([P, N], BF16, name=f"qb{h}", tag=f"qb{h}") for h in range(HL)]
        for h in range(HL):
            qbs = ps.tile([P, N], F32, name="ps_t", tag="ps_t")
            nc.tensor.matmul(
                qbs[:], lhsT=ones_t[:], rhs=qrow[h][:], start=True, stop=True
            )
            nc.scalar.copy(qb[h][:], qbs[:])

        # q^T columns (q_i per partition) via PE transpose: qt[it][:, h]
        qt = [const.tile([P, HL], F32, name=f"qt{it}", tag=f"qt{it}") for it in range(2)]
        for it in range(2):
            tpb = ps.tile([P, N], BF16, name="ps_t", tag="ps_t")
            nc.tensor.transpose(
                tpb[:, :HL], q_sb[:, it * P : (it + 1) * P], identb[:HL, :HL]
            )
            nc.scalar.copy(qt[it][:], tpb[:, :HL])

        # ---------------- M = mask ? ln(A0+1e-8) : -1e30 ------------
        mtile = [const.tile([P, N], BF16, name=f"mtile{it}", tag=f"mtile{it}") for it in range(2)]
        for it in range(2):
            nc.gpsimd.memset(mtile[it][:], -1e30)
            lna = work.tile([P, N], BF16, name="lna", tag="lna")
            nc.scalar.activation(lna[:], a0t[it][:], AF.Ln, bias=eps_col[:])
            nc.vector.copy_predicated(mtile[it][:], mskt[it][:], lna[:])
        # pre-sum mask-bias and key-side q so the softmax tail does one add
        mq = [
            [const.tile([P, N], BF16, name=f"mq{h}_{it}", tag=f"mq{h}_{it}") for it in range(2)]
            for h in range(HL)
        ]
        for h in range(HL):
            for it in range(2):
                nc.vector.tensor_tensor(
                    out=mq[h][it][:], in0=mtile[it][:], in1=qb[h][:], op=ALU.add
                )

        # ------- pairwise relu pass + PE reduce + per-half tail ------
        # Two independent phases (query halves it=0,1): fills 2it,2it+1
        # then that half's softmax/AV/projection, so the second half's
        # relu pass overlaps the first half's tail work.
        e_raw0 = [
            const.tile([P, N], BF16, name=f"e_raw0_{h}", tag=f"e_raw0_{h}")
            for h in range(HL)
        ]
        e_raw1 = [
            const.tile([P, P], BF16, name=f"e_raw1_{h}", tag=f"e_raw1_{h}")
            for h in range(HL)
        ]
        pt = [
            [const.tile([P, N], BF16, name=f"pt{h}_{it}", tag=f"pt{h}_{it}") for it in range(2)]
            for h in range(HL)
        ]
        rec = [
            [const.tile([P, 1], F32, name=f"rec{h}_{it}", tag=f"rec{h}_{it}") for it in range(2)]
            for h in range(HL)
        ]
        att = [
            [const.tile([P, N], BF16, name=f"att{h}_{jh}", tag=f"att{h}_{jh}") for jh in range(2)]
            for h in range(HL)
        ]
        ytile = [const.tile([P, P], BF16, name=f"ytile{ib}", tag=f"ytile{ib}") for ib in range(2)]
        yt = const.tile([P, N], BF16, name="yt", tag="yt")

        for it in range(2):
            # Phase it=1 generates only the j>=128 half: the (i>=128, j<128)
            # quadrant of the symmetric relu-score equals the transpose of
            # phase 0's (i<128, j>=128) quadrant (PE-transposed below).
            jw = N if it == 0 else P
            j0 = N - jw
            for G in (2 * it, 2 * it + 1):
                fps = fillps.tile([P, 2 * jw], F32, name="fill", tag="fill")
                for c in range(32):
                    st = spool.tile([P, 2 * jw], BF16, name="st", tag="st")
                    for half in range(2):
                        i = 64 * G + 32 * half + c
                        dst = st[:, half * jw : (half + 1) * jw]
                        if _gen_engine(c) == "act":
                            nc.scalar.activation(
                                dst,
                                xtb[:, j0:N],
                                AF.Relu,
                                bias=xtf[:, i : i + 1],
                            )
                        else:
                            nc.vector.tensor_scalar(
                                out=dst,
                                in0=xtb[:, j0:N],
                                scalar1=xtf[:, i : i + 1],
                                scalar2=0.0,
                                op0=ALU.add,
                                op1=ALU.max,
                            )
                    nc.tensor.matmul(
                        fps[:],
                        lhsT=zt[:, DH - c : 160 - c],
                        rhs=st[:],
                        start=(c == 0),
                        stop=(c == 31),
                    )
                dr = drpool.tile([P, 2 * jw], BF16, name="dr", tag="dr")
                nc.scalar.copy(dr[:], fps[:])
                for h in range(HL):
                    for half in range(2):
                        r0 = (64 * G + 32 * half) % P
                        dst = (
                            e_raw0[h][r0 : r0 + 32, :]
                            if it == 0
                            else e_raw1[h][r0 : r0 + 32, :]
                        )
                        nc.sync.dma_start(
                            out=dst,
                            in_=dr[h * DH : (h + 1) * DH, half * jw : (half + 1) * jw],
                        )

            # softmax for this query half (unnormalized exp + rowsum)
            for h in range(HL):
                if it == 0:
                    e3 = work.tile([P, N], BF16, name="e3", tag="e3")
                    nc.vector.tensor_tensor(
                        out=e3[:], in0=e_raw0[h][:], in1=mq[h][0][:], op=ALU.add
                    )
                    esrc = e3
                else:
                    # j>=128 half: own fill + mask/q_j bias
                    e3 = work.tile([P, N], BF16, name="e3", tag="e3")
                    nc.vector.tensor_tensor(
                        out=e3[:, P:N],
                        in0=e_raw1[h][:],
                        in1=mq[h][1][:, P:N],
                        op=ALU.add,
                    )
                    # j<128 half: transpose of phase 0's (i<128, j>=128)
                    # quadrant, bias fused into the PSUM drain
                    tpb = ps.tile([P, N], BF16, name="ps_t", tag="ps_t")
                    nc.tensor.transpose(tpb[:, :P], e_raw0[h][:, P:N], identb[:])
                    nc.vector.tensor_tensor(
                        out=e3[:, 0:P],
                        in0=tpb[:, :P],
                        in1=mq[h][1][:, 0:P],
                        op=ALU.add,
                    )
                    esrc = e3
                den = work.tile([P, 1], F32, name="den", tag="den")
                nc.scalar.activation(
                    pt[h][it][:],
                    esrc[:],
                    AF.Exp,
                    bias=qt[it][:, h : h + 1],
                    accum_out=den[:],
                )
                nc.vector.reciprocal(rec[h][it][:], den[:])

            # attn^T via PE for this half
            for h in range(HL):
                for jh in range(2):
                    tpb = ps.tile([P, N], BF16, name="ps_t", tag="ps_t")
                    nc.tensor.transpose(
                        tpb[:, :P], pt[h][it][:, jh * P : (jh + 1) * P], identb[:]
                    )
                    if (h + jh) % 2 == 0:
                        nc.scalar.copy(att[h][jh][:, it * P : (it + 1) * P], tpb[:, :P])
                    else:
                        nc.vector.tensor_copy(
                            att[h][jh][:, it * P : (it + 1) * P], tpb[:, :P]
                        )

            # AV + 1/den scale for i-block it
            ib = it
            for h in range(HL):
                yps = ps.tile([P, DH], F32, name="ps_y", tag="ps_t")
                for k in range(2):
                    nc.tensor.matmul(
                        yps[:],
                        lhsT=att[h][k][:, ib * P : (ib + 1) * P],
                        rhs=xpb[k][:, h * DH : (h + 1) * DH],
                        start=(k == 0),
                        stop=(k == 1),
                    )
                nc.vector.tensor_scalar(
                    out=ytile[ib][:, h * DH : (h + 1) * DH],
                    in0=yps[:],
                    scalar1=rec[h][ib][:],
                    scalar2=None,
                    op0=ALU.mult,
                )

            # out rows for this i-block: transpose Y then @ WoutG
            tpb = ps.tile([P, N], BF16, name="ps_t", tag="ps_t")
            nc.tensor.transpose(tpb[:, :P], ytile[ib][:], identb[:])
            nc.scalar.copy(yt[:, ib * P : (ib + 1) * P], tpb[:, :P])
            ops_ = ps.tile([P, N], F32, name="ps_t", tag="ps_t")
            nc.tensor.matmul(
                ops_[:],
                lhsT=yt[:, ib * P : (ib + 1) * P],
                rhs=wotb[:],
                start=True,
                stop=True,
            )
            osb = work.tile([P, N], F32, name="osb", tag="osb")
            nc.scalar.copy(osb[:], ops_[:])
            nc.sync.dma_start(out=out_d[ib * P : (ib + 1) * P, :], in_=osb[:])


_NC_CACHE = None


def _get_module():
    global _NC_CACHE
    if _NC_CACHE is None:
        nc = build_module()
        _split_multiwait(nc)  # HW-compile only; breaks CoreSim bookkeeping
        _NC_CACHE = nc
    return _NC_CACHE


def make_in_maps(H, mask, A0, W_lin, a, W_out):
    H = np.ascontiguousarray(np.asarray(H, dtype=np.float32))
    W_lin = np.ascontiguousarray(np.asarray(W_lin, dtype=np.float32))
    W_out = np.ascontiguousarray(np.asarray(W_out, dtype=np.float32))
    a = np.ascontiguousarray(np.asarray(a, dtype=np.float32))
    A0 = np.ascontiguousarray(np.asarray(A0, dtype=np.float32))
    mask_u8 = np.ascontiguousarray(np.asarray(mask).astype(np.uint8))
    in_maps = []
    for c in range(NCORES):
        b, g = divmod(c, 2)
        in_maps.append(
            {
                "Hb": H[b],
                "WlinG": np.ascontiguousarray(W_lin[:, g * P : (g + 1) * P]),
                "WoutG": np.ascontiguousarray(W_out[g * P : (g + 1) * P, :]),
                "aG": np.ascontiguousarray(a[g * HL : (g + 1) * HL, :]),
                "mask": mask_u8,
                "A0": A0,
            }
        )
    return in_maps


def run_raw(H, mask, A0, W_lin, a, W_out, **kw):
    nc = _get_module()
    in_maps = make_in_maps(H, mask, A0, W_lin, a, W_out)
    return run_bass_kernel_spmd(nc, in_maps, list(range(NCORES)), **kw)


def assemble(results):
    parts = [results[c]["out"] for c in range(NCORES)]
    out = np.stack(
        [parts[2 * b].astype(np.float32) + parts[2 * b + 1] for b in range(4)]
    )
    return out.astype(np.float32)


def kernel(H, mask, A0, W_lin, a, W_out):
    res = run_raw(H, mask, A0, W_lin, a, W_out)
    return assemble(res.results)


# revision 9
# speedup vs baseline: 1.0152x; 1.0152x over previous
"""GATv2 layer on 8 Trainium2 NeuronCores.

Problem (hardcoded): B=4, N=256, D=256, HEADS=8, DH=32, neg_slope=0.2.

    X = (H @ W_lin) split into heads               [B, h, N, 32]
    e = leaky_relu(Xi + Xj, 0.2) . a[h]            [B, h, N, N]
    e += ln(A0 + 1e-8);  e = -inf outside mask
    attn = softmax_j(e);  Y = attn @ X  (heads merged) @ W_out

Sharding: 8 cores = (batch b = core//2) x (head-group g = core%2, 4 heads
each).  Every core computes a full [N, D] partial of Y[b] (its 4 heads'
contribution through W_out rows g*128:(g+1)*128); host sums the two
partials per batch.  SPMD: all cores run the same program on pre-sliced
inputs (no partition-id branching).

Math trick: leaky(x) = 0.2*x + 0.8*relu(x), so with q = 0.2 * a^T X:

    e[h,i,j] = 0.8 * sum_d a[h,d]*relu(X[h,d,i]+X[h,d,j]) + q[h,i] + q[h,j]

The pairwise relu pass packs all 4 local heads' dims on the 128 SBUF
partitions (Xt[(h,d), i]) and is a single fused op per query i
(DVE tensor_scalar(add,max0) or ACT Relu with per-partition bias).  The
d-reduction is a PE matmul with a sliding-window view of a zero-padded
block-diagonal 0.8*a weight matrix, accumulating rows 4c+h for 32
query nodes c into one [128, 512] PSUM tile (PE requires out base
partition 32-aligned, so zero columns of the window produce +0 rows).

The score pipeline runs in bf16: the PE fill already used fp32r HIGH
(single-pass, bf16-precision multiplies), so bf16 storage costs no
accuracy while unlocking DVE 4x tensor_scalar, ACT 2x, FWL weight
loads, and 1 cycle/row PE streaming on the transpose/AV matmuls.
X itself is computed in full fp32 (LOW_HIGH) then rounded once.
"""

import numpy as np

try:
    import concourse.bass as bass
except ImportError:  # pragma: no cover - fallback for bare containers
    import sys

    sys.path.insert(0, "/opt/trn_rl_repo")
    import concourse.bass as bass

import concourse.mybir as mybir
import concourse.tile as tile
from concourse import masks
from concourse.bass_utils import run_bass_kernel_spmd

F32 = mybir.dt.float32
BF16 = mybir.dt.bfloat16
U8 = mybir.dt.uint8
AF = mybir.ActivationFunctionType
ALU = mybir.AluOpType

N = 256
D = 256
HEADS = 8
DH = 32
HL = 4  # heads per core
P = 128
NCORES = 8

# Per-c engine assignment for the pairwise relu pass.  bf16 costs:
# DVE tensor_scalar 4x ~127ns, ACT Relu 2x ~293ns (free=256), so ACT
# carries ~10/32 of the c values given its other work (exp, drains).
_ACT_C = {0, 3, 6, 10, 13, 16, 20, 23, 26, 29}


def _gen_engine(c):
    if c in _ACT_C:
        return "act"
    return "dve"


def _split_multiwait(nc, maxw=1):
    """Walrus codegen here rejects instructions with >1 sem wait ("Too many
    sync wait commands", CoreV3GenImpl setupSyncWait).  Tile's kernel-tail
    drain carries one wait per ticked processor; hoist the extras into
    single-wait NoOps on the same engine just before the instruction."""
    import bass_rust

    n = 0
    for f in nc.m.functions:
        for b in f.blocks:
            new, changed = [], False
            for i in b.instructions:
                si = i.sync_info
                ow = list(si.on_wait) if (si is not None and si.on_wait) else []
                if len(ow) > maxw:
                    extra, keep = ow[:-maxw], ow[-maxw:]
                    for w in extra:
                        nop = mybir.InstNoOp(name=f"I-waitsplit-{n}")
                        n += 1
                        nop.engine = i.engine
                        nop.sync_info = bass_rust.SyncInfo(on_wait=[w], on_update=[])
                        new.append(nop)
                    i.sync_info = bass_rust.SyncInfo(
                        on_wait=keep,
                        on_update=list(si.on_update) if si.on_update else [],
                    )
                    changed = True
                new.append(i)
            if changed:
                b.instructions = new


def build_module():
    nc = bass.Bass("TRN2", target_bir_lowering=False, debug=False)

    hb = nc.dram_tensor("Hb", [N, D], F32, kind="ExternalInput").ap()
    wlg = nc.dram_tensor("WlinG", [D, P], F32, kind="ExternalInput").ap()
    wog = nc.dram_tensor("WoutG", [P, D], F32, kind="ExternalInput").ap()
    ag = nc.dram_tensor("aG", [HL, DH], F32, kind="ExternalInput").ap()
    mask_d = nc.dram_tensor("mask", [N, N], U8, kind="ExternalInput").ap()
    a0_d = nc.dram_tensor("A0", [N, N], F32, kind="ExternalInput").ap()
    out_d = nc.dram_tensor("out", [N, D], F32, kind="ExternalOutput").ap()

    with tile.TileContext(nc) as tc:
        _body(nc, tc, hb, wlg, wog, ag, mask_d, a0_d, out_d)
    return nc


def _body(nc, tc, hb, wlg, wog, ag, mask_d, a0_d, out_d):
    from contextlib import ExitStack

    ctx = ExitStack()
    with ctx:
        const = ctx.enter_context(tc.tile_pool(name="const", bufs=1))
        work = ctx.enter_context(tc.tile_pool(name="work", bufs=3))
        spool = ctx.enter_context(tc.tile_pool(name="spool", bufs=12))
        drpool = ctx.enter_context(tc.tile_pool(name="drpool", bufs=3))
        ps = ctx.enter_context(tc.tile_pool(name="ps", bufs=4, space="PSUM"))
        fillps = ctx.enter_context(tc.tile_pool(name="fillps", bufs=3, space="PSUM"))

        # ---------------- setup: loads -------------------------------
        ident = const.tile([P, P], F32, name="ident", tag="ident")
        masks.make_identity(nc, ident[:])
        identb = const.tile([P, P], BF16, name="identb", tag="identb")
        nc.vector.tensor_copy(identb[:], ident[:])

        # HAM warmup: PE sits idle during the input DMAs, which leaves the
        # clock gate at 4/8 (1.2 GHz) for the first ~3.4us of real matmuls.
        # Nine F=128 fp32 matmuls (~430ns each cold) burn the activity
        # window so the fill pipeline starts at 8/8.
        for k in range(9):
            wrm = fillps.tile([P, 2 * N], F32, name="wrm", tag="fill")
            nc.tensor.matmul(
                wrm[:, :P], lhsT=ident[:], rhs=ident[:], start=True, stop=True
            )

        hbt = [const.tile([P, D], F32, name=f"hbt{k}", tag=f"hbt{k}") for k in range(2)]
        for k in range(2):
            nc.sync.dma_start(out=hbt[k][:], in_=hb[k * P : (k + 1) * P, :])
        wlt = [const.tile([P, P], F32, name=f"wlt{k}", tag=f"wlt{k}") for k in range(2)]
        for k in range(2):
            nc.sync.dma_start(out=wlt[k][:], in_=wlg[k * P : (k + 1) * P, :])
        wot = const.tile([P, D], F32, name="wot", tag="wot")
        nc.sync.dma_start(out=wot[:], in_=wog[:, :])
        mskt = [const.tile([P, N], U8, name=f"mskt{k}", tag=f"mskt{k}") for k in range(2)]
        a0t = [const.tile([P, N], F32, name=f"a0t{k}", tag=f"a0t{k}") for k in range(2)]
        for k in range(2):
            nc.sync.dma_start(out=mskt[k][:], in_=mask_d[k * P : (k + 1) * P, :])
            nc.sync.dma_start(out=a0t[k][:], in_=a0_d[k * P : (k + 1) * P, :])

        # Zbig: [128, 192] zeros with 0.8*aG[h] block at rows h*32, col 32+32h.
        # Sliding window Zbig[:, 32-c : 160-c] as matmul lhsT puts head h's
        # reduction of query c at output partition h*32 + c.
        # Ablk: [128, 4] blockdiag(a) for the q matmul
        ablk = const.tile([P, HL], F32, name="ablk", tag="ablk")
        nc.gpsimd.memset(ablk[:], 0.0)
        for h in range(HL):
            nc.sync.dma_start(
                out=ablk[h * DH : (h + 1) * DH, h : h + 1],
                in_=ag[h : h + 1, :],
            )
        ablkb = const.tile([P, HL], BF16, name="ablkb", tag="ablkb")
        nc.vector.tensor_copy(ablkb[:], ablk[:])
        zt = const.tile([P, 192], BF16, name="zt", tag="zt")
        nc.gpsimd.memset(zt[:], 0.0)
        nc.vector.tensor_scalar(
            out=zt[:, DH : DH + HL * DH : DH],
            in0=ablkb[:],
            scalar1=0.8,
            scalar2=None,
            op0=ALU.mult,
        )

        wotb = const.tile([P, D], BF16, name="wotb", tag="wotb")
        nc.vector.tensor_copy(wotb[:], wot[:])

        ones_t = const.tile([1, P], F32, name="ones_t", tag="ones_t")
        nc.gpsimd.memset(ones_t[:], 1.0)
        eps_col = const.tile([P, 1], F32, name="eps_col", tag="eps_col")
        nc.gpsimd.memset(eps_col[:], 1e-8)

        # ---------------- HT = Hb^T, Xp = Hb @ WlinG (fp32), Xt bf16 -
        ht = [const.tile([P, N], F32, name=f"ht{k}", tag=f"ht{k}") for k in range(2)]
        for cb in range(2):  # column block of Hb = partition block of HT
            for ib in range(2):
                tp = ps.tile([P, N], F32, name="ps_t", tag="ps_t")
                nc.tensor.transpose(
                    tp[:, :P], hbt[ib][:, cb * P : (cb + 1) * P], ident[:]
                )
                nc.scalar.copy(ht[cb][:, ib * P : (ib + 1) * P], tp[:, :P])

        xpb = [const.tile([P, P], BF16, name=f"xpb{ib}", tag=f"xpb{ib}") for ib in range(2)]
        for ib in range(2):
            xps = ps.tile([P, N], F32, name="ps_t", tag="ps_t")
            for k in range(2):
                nc.tensor.matmul(
                    xps[:, :P],
                    lhsT=ht[k][:, ib * P : (ib + 1) * P],
                    rhs=wlt[k][:],
                    start=(k == 0),
                    stop=(k == 1),
                )
            nc.scalar.copy(xpb[ib][:], xps[:, :P])

        xtb = const.tile([P, N], BF16, name="xtb", tag="xtb")
        for ib in range(2):
            tpb = ps.tile([P, N], BF16, name="ps_t", tag="ps_t")
            nc.tensor.transpose(tpb[:, :P], xpb[ib][:], identb[:])
            nc.scalar.copy(xtb[:, ib * P : (ib + 1) * P], tpb[:, :P])

        # fp32 image of X^T: per-partition scalar operands (DVE scalar1 /
        # ACT bias) must be fp32; values are identical to the bf16 xtb.
        xtf = const.tile([P, N], F32, name="xtf", tag="xtf")
        nc.vector.tensor_copy(xtf[:], xtb[:])

        # ---------------- q = 0.2 * a^T X  --------------------------
        qps = ps.tile([HL, N], F32, name="ps_q", tag="ps_t")
        nc.tensor.matmul(qps[:], lhsT=ablkb[:], rhs=xtb[:], start=True, stop=True)
        q_sb = const.tile([HL, N], F32, name="q_sb", tag="q_sb")
        nc.scalar.activation(q_sb[:], qps[:], AF.Copy, bias=0.0, scale=0.2)

        # q broadcast along partitions (q_j along free), per head
        qrow = [const.tile([1, N], F32, name=f"qrow{h}", tag=f"qrow{h}") for h in range(HL)]
        for h in range(HL):
            nc.sync.dma_start(out=qrow[h][:], in_=q_sb[h : h + 1, :])
        qb = [const.tile([P, N], F32, name=f"qb{h}", tag=f"qb{h}") for h in range(HL)]
        for h in range(HL):
            qbs = ps.tile([P, N], F32, name="ps_t", tag="ps_t")
            nc.tensor.matmul(
                qbs[:], lhsT=ones_t[:], rhs=qrow[h][:], start=True, stop=True
            )
            nc.scalar.copy(qb[h][:], qbs[:])

        # q^T columns (q_i per partition) via PE transpose: qt[it][:, h]
        qt = [const.tile([P, HL], F32, name=f"qt{it}", tag=f"qt{it}") for it in range(2)]
        for it in range(2):
            tpq = ps.tile([P, N], F32, name="ps_t", tag="ps_t")
            nc.tensor.transpose(
                tpq[:, :HL], q_sb[:, it * P : (it + 1) * P], ident[:HL, :HL]
            )
            nc.scalar.copy(qt[it][:], tpq[:, :HL])

        # ---------------- M = mask ? ln(A0+1e-8) : -1e30 ------------
        mtile = [const.tile([P, N], F32, name=f"mtile{it}", tag=f"mtile{it}") for it in range(2)]
        for it in range(2):
            nc.gpsimd.memset(mtile[it][:], -1e30)
            lna = work.tile([P, N], F32, name="lna", tag="lna")
            nc.scalar.activation(lna[:], a0t[it][:], AF.Ln, bias=eps_col[:])
            nc.vector.copy_predicated(mtile[it][:], mskt[it][:], lna[:])
        # pre-sum mask-bias and key-side q so the softmax tail does one add
        mq = [
            [const.tile([P, N], F32, name=f"mq{h}_{it}", tag=f"mq{h}_{it}") for it in range(2)]
            for h in range(HL)
        ]
        for h in range(HL):
            for it in range(2):
                nc.vector.tensor_tensor(
                    out=mq[h][it][:], in0=mtile[it][:], in1=qb[h][:], op=ALU.add
                )

        # ------- pairwise relu pass + PE reduce + per-half tail ------
        # Two independent phases (query halves it=0,1): fills 2it,2it+1
        # then that half's softmax/AV/projection, so the second half's
        # relu pass overlaps the first half's tail work.
        e_raw0 = [
            const.tile([P, N], F32, name=f"e_raw0_{h}", tag=f"e_raw0_{h}")
            for h in range(HL)
        ]
        e_raw1 = [
            const.tile([P, P], F32, name=f"e_raw1_{h}", tag=f"e_raw1_{h}")
            for h in range(HL)
        ]
        pt = [
            [const.tile([P, N], BF16, name=f"pt{h}_{it}", tag=f"pt{h}_{it}") for it in range(2)]
            for h in range(HL)
        ]
        rec = [
            [const.tile([P, 1], F32, name=f"rec{h}_{it}", tag=f"rec{h}_{it}") for it in range(2)]
            for h in range(HL)
        ]
        att = [
            [const.tile([P, N], BF16, name=f"att{h}_{jh}", tag=f"att{h}_{jh}") for jh in range(2)]
            for h in range(HL)
        ]
        ytile = [const.tile([P, P], BF16, name=f"ytile{ib}", tag=f"ytile{ib}") for ib in range(2)]
        yt = const.tile([P, N], BF16, name="yt", tag="yt")

        for it in range(2):
            # Phase it=1 generates only the j>=128 half: the (i>=128, j<128)
            # quadrant of the symmetric relu-score equals the transpose of
            # phase 0's (i<128, j>=128) quadrant (PE-transposed below).
            jw = N if it == 0 else P
            j0 = N - jw
            for G in (2 * it, 2 * it + 1):
                fps = fillps.tile([P, 2 * jw], F32, name="fill", tag="fill")
                for c in range(32):
                    st = spool.tile([P, 2 * jw], BF16, name="st", tag="st")
                    for half in range(2):
                        i = 64 * G + 32 * half + c
                        dst = st[:, half * jw : (half + 1) * jw]
                        if _gen_engine(c) == "act":
                            nc.scalar.activation(
                                dst,
                                xtb[:, j0:N],
                                AF.Relu,
                                bias=xtf[:, i : i + 1],
                            )
                        else:
                            nc.vector.tensor_scalar(
                                out=dst,
                                in0=xtb[:, j0:N],
                                scalar1=xtf[:, i : i + 1],
                                scalar2=0.0,
                                op0=ALU.add,
                                op1=ALU.max,
                            )
                    nc.tensor.matmul(
                        fps[:],
                        lhsT=zt[:, DH - c : 160 - c],
                        rhs=st[:],
                        start=(c == 0),
                        stop=(c == 31),
                    )
                dr = drpool.tile([P, 2 * jw], F32, name="dr", tag="dr")
                nc.scalar.copy(dr[:], fps[:])
                for h in range(HL):
                    for half in range(2):
                        r0 = (64 * G + 32 * half) % P
                        dst = (
                            e_raw0[h][r0 : r0 + 32, :]
                            if it == 0
                            else e_raw1[h][r0 : r0 + 32, :]
                        )
                        nc.sync.dma_start(
                            out=dst,
                            in_=dr[h * DH : (h + 1) * DH, half * jw : (half + 1) * jw],
                        )

            # softmax for this query half (unnormalized exp + rowsum)
            for h in range(HL):
                if it == 0:
                    e3 = work.tile([P, N], F32, name="e3", tag="e3")
                    nc.vector.tensor_tensor(
                        out=e3[:], in0=e_raw0[h][:], in1=mq[h][0][:], op=ALU.add
                    )
                    esrc = e3
                else:
                    # j>=128 half: own fill + mask/q_j bias
                    e3 = work.tile([P, N], F32, name="e3", tag="e3")
                    nc.vector.tensor_tensor(
                        out=e3[:, P:N],
                        in0=e_raw1[h][:],
                        in1=mq[h][1][:, P:N],
                        op=ALU.add,
                    )
                    # j<128 half: transpose of phase 0's (i<128, j>=128)
                    # quadrant, bias fused into the PSUM drain
                    tpe = ps.tile([P, N], F32, name="ps_t", tag="ps_t")
                    nc.tensor.transpose(tpe[:, :P], e_raw0[h][:, P:N], ident[:])
                    nc.vector.tensor_tensor(
                        out=e3[:, 0:P],
                        in0=tpe[:, :P],
                        in1=mq[h][1][:, 0:P],
                        op=ALU.add,
                    )
                    esrc = e3
                den = work.tile([P, 1], F32, name="den", tag="den")
                nc.scalar.activation(
                    pt[h][it][:],
                    esrc[:],
                    AF.Exp,
                    bias=qt[it][:, h : h + 1],
                    accum_out=den[:],
                )
                nc.vector.reciprocal(rec[h][it][:], den[:])

            # attn^T via PE for this half
            for h in range(HL):
                for jh in range(2):
                    tpb = ps.tile([P, N], BF16, name="ps_t", tag="ps_t")
                    nc.tensor.transpose(
                        tpb[:, :P], pt[h][it][:, jh * P : (jh + 1) * P], identb[:]
                    )
                    if (h + jh) % 2 == 0:
                        nc.scalar.copy(att[h][jh][:, it * P : (it + 1) * P], tpb[:, :P])
                    else:
                        nc.vector.tensor_copy(
                            att[h][jh][:, it * P : (it + 1) * P], tpb[:, :P]
                        )

            # AV + 1/den scale for i-block it
            ib = it
            for h in range(HL):
                yps = ps.tile([P, DH], F32, name="ps_y", tag="ps_t")
                for k in range(2):
                    nc.tensor.matmul(
                        yps[:],
                        lhsT=att[h][k][:, ib * P : (ib + 1) * P],
                        rhs=xpb[k][:, h * DH : (h + 1) * DH],
                        start=(k == 0),
                        stop=(k == 1),
                    )
                nc.vector.tensor_scalar(
                    out=ytile[ib][:, h * DH : (h + 1) * DH],
                    in0=yps[:],
                    scalar1=rec[h][ib][:],
                    scalar2=None,
                    op0=ALU.mult,
                )

            # out rows for this i-block: transpose Y then @ WoutG
            tpb = ps.tile([P, N], BF16, name="ps_t", tag="ps_t")
            nc.tensor.transpose(tpb[:, :P], ytile[ib][:], identb[:])
            nc.scalar.copy(yt[:, ib * P : (ib + 1) * P], tpb[:, :P])
            ops_ = ps.tile([P, N], F32, name="ps_t", tag="ps_t")
            nc.tensor.matmul(
                ops_[:],
                lhsT=yt[:, ib * P : (ib + 1) * P],
                rhs=wotb[:],
                start=True,
                stop=True,
            )
            osb = work.tile([P, N], F32, name="osb", tag="osb")
            nc.scalar.copy(osb[:], ops_[:])
            nc.sync.dma_start(out=out_d[ib * P : (ib + 1) * P, :], in_=osb[:])


_NC_CACHE = None


def _get_module():
    global _NC_CACHE
    if _NC_CACHE is None:
        nc = build_module()
        _split_multiwait(nc)  # HW-compile only; breaks CoreSim bookkeeping
        _NC_CACHE = nc
    return _NC_CACHE


def make_in_maps(H, mask, A0, W_lin, a, W_out):
    H = np.ascontiguousarray(np.asarray(H, dtype=np.float32))
    W_lin = np.ascontiguousarray(np.asarray(W_lin, dtype=np.float32))
    W_out = np.ascontiguousarray(np.asarray(W_out, dtype=np.float32))
    a = np.ascontiguousarray(np.asarray(a, dtype=np.float32))
    A0 = np.ascontiguousarray(np.asarray(A0, dtype=np.float32))
    mask_u8 = np.ascontiguousarray(np.asarray(mask).astype(np.uint8))
    in_maps = []
    for c in range(NCORES):
        b, g = divmod(c, 2)
        in_maps.append(
            {
                "Hb": H[b],
                "WlinG": np.ascontiguousarray(W_lin[:, g * P : (g + 1) * P]),
                "WoutG": np.ascontiguousarray(W_out[g * P : (g + 1) * P, :]),
                "aG": np.ascontiguousarray(a[g * HL : (g + 1) * HL, :]),
                "mask": mask_u8,
                "A0": A0,
            }
        )
    return in_maps


def run_raw(H, mask, A0, W_lin, a, W_out, **kw):
    nc = _get_module()
    in_maps = make_in_maps(H, mask, A0, W_lin, a, W_out)
    return run_bass_kernel_spmd(nc, in_maps, list(range(NCORES)), **kw)


def assemble(results):
    parts = [results[c]["out"] for c in range(NCORES)]
    out = np.stack(
        [parts[2 * b].astype(np.float32) + parts[2 * b + 1] for b in range(4)]
    )
    return out.astype(np.float32)


def kernel(H, mask, A0, W_lin, a, W_out):
    res = run_raw(H, mask, A0, W_lin, a, W_out)
    return assemble(res.results)
